# revision 47
# baseline (speedup 1.0000x reference)
"""MinCutPool forward on 8 trn2 NeuronCores.

Shapes: emb [4,4096,128], adj [4,4096,4096], logits_param [4096,32],
gumbel_noise [4,4096,32].

Sharding: core c handles batch b=c//2 and adj column-half h=c%2.

Math (per batch b, with L = [s_sample | s_soft | q | ones]  [N, 66]):
    T66 = L^T @ adj[:, cols]            # [66, 2048]  contraction over rows n
      rows 0:32  = s_sample^T A  (T_sa)
      rows 32:64 = s_soft^T A    (T_ss)
      row  64    = q^T A = r     (q[n] = sum_k s_soft[n,k]^2)
      row  65    = ones^T A      (unused)
    out2 = L_local^T @ T66^T            # [66, 66]  contraction over local cols m
      out2[0:32,0:32][l,k]   = G[k,l] partial   (G = s_sample^T A s_sample)
      trace(out2[32:64,32:64]) = mincut_num partial
      out2[65,64]            = mincut_den partial
    out_emb = s_sample^T @ emb          # rows 0:32 of L^T @ emb
    ss = s_soft^T s_soft                # for ortho_loss

Host finalizes the tiny [K,K]-level math (pair-sums, normalization, losses).
"""

import os
import sys

sys.path.insert(0, "/opt/trn_rl_repo")

import numpy as np

B, N, F, K = 4, 4096, 128, 32
P = 128                  # partitions
NB = N // P              # 32 row blocks
MH = N // 2              # 2048 local adj columns per core
MB = MH // 512           # 4 T-matmul output column blocks
JB = MH // P             # 16 stage-2 blocks
L = 2 * K + 2            # 66 fused weight columns
EPS = 1e-8
T1 = max(1.0 * 0.99995, 0.05)
T2 = max(T1 * 0.99995, 0.05)

USE_F32R = os.environ.get("KERNEL_F32R", "1") == "1"

_cache = {}


def _build_program():
    from contextlib import ExitStack

    import concourse.bass as bass
    import concourse.bacc as bacc
    import concourse.tile as tile
    from concourse import mybir
    from concourse.masks import make_identity

    f32 = mybir.dt.float32
    f32r = mybir.dt.float32r
    AX = mybir.AxisListType
    OP = mybir.AluOpType
    ACT = mybir.ActivationFunctionType

    nc = bacc.Bacc("TRN2", target_bir_lowering=False)

    adj_dt = f32r if USE_F32R else f32
    adj_cols = nc.dram_tensor("adj_cols", [N, MH], adj_dt, kind="ExternalInput")
    emb_in = nc.dram_tensor("emb_in", [MH, F], adj_dt, kind="ExternalInput")
    # host pre-permutes these to [p, nb, k] so the staging DMA is contiguous
    z_in = nc.dram_tensor("z_in", [P, NB * K], f32, kind="ExternalInput")
    logits_t2_in = nc.dram_tensor("logits_t2", [P, NB * K], f32, kind="ExternalInput")

    s_sample_o = nc.dram_tensor("s_sample_o", [N, K], f32, kind="ExternalOutput")
    out_emb_a_o = nc.dram_tensor("out_emb_a_o", [K, F], f32, kind="ExternalOutput")
    out_emb_b_o = nc.dram_tensor("out_emb_b_o", [K, F], f32, kind="ExternalOutput")
    out2a_o = nc.dram_tensor("out2a_o", [L, L], f32, kind="ExternalOutput")
    out2b_o = nc.dram_tensor("out2b_o", [L, L], f32, kind="ExternalOutput")
    ss_o = nc.dram_tensor("ss_o", [K, K], f32, kind="ExternalOutput")

    def mmdt(ap):
        return ap.bitcast(f32r) if USE_F32R else ap

    with tile.TileContext(nc) as tc, ExitStack() as ctx:
        consts = ctx.enter_context(tc.tile_pool(name="consts", bufs=1))
        stage_in = ctx.enter_context(tc.tile_pool(name="stage_in", bufs=1))
        l66p = ctx.enter_context(tc.tile_pool(name="l66", bufs=NB))
        scr = ctx.enter_context(tc.tile_pool(name="scr", bufs=8))
        col = ctx.enter_context(tc.tile_pool(name="col", bufs=12))
        adjp = ctx.enter_context(tc.tile_pool(name="adjp", bufs=4))
        l66rp = ctx.enter_context(tc.tile_pool(name="l66r", bufs=NB))
        embp = ctx.enter_context(tc.tile_pool(name="embp", bufs=JB))
        t66p = ctx.enter_context(tc.tile_pool(name="t66sb", bufs=1))
        trp = ctx.enter_context(tc.tile_pool(name="trsb", bufs=3))
        outp = ctx.enter_context(tc.tile_pool(name="outp", bufs=1))

        # iota_desc[p, k] = K - k  (32..1), same on every partition: used to
        # pick the FIRST argmax on ties, matching jnp.argmax. Generated on
        # gpsimd, then bounced through a DVE copy so the hot DVE consumer's
        # producer is same-engine (keeps per-instruction wait counts low).
        iota_g = consts.tile([P, K], f32, tag="iota_g")
        nc.gpsimd.iota(
            iota_g[:, :],
            pattern=[[-1, K]],
            base=K,
            channel_multiplier=0,
            allow_small_or_imprecise_dtypes=True,
        )
        iota_desc = consts.tile([P, K], f32, tag="iota_desc")
        nc.vector.tensor_copy(iota_desc[:, :], iota_g[:, :])
        # ACT-produced zero bias (scale=0 copy) so Exp's bias needs no
        # cross-engine wait.
        zero_bias = consts.tile([P, 1], f32, tag="zero_bias")
        nc.scalar.mul(zero_bias[:, :], zero_bias[:, :], 0.0)
        ident_g = consts.tile([L, L], f32, tag="ident_g")
        make_identity(nc, ident_g[:, :])
        identity66 = consts.tile([L, L], f32, tag="identity66")
        nc.vector.tensor_copy(identity66[:, :], ident_g[:, :])

        # z = logits/t1 + gumbel (host-computed) and logits/t2, already
        # host-permuted to [p, nb, k]. Loaded in per-2-block chunks on the
        # gpsimd SWDGE ring (HWDGE rings are reserved for the adj stream);
        # small chunks mean block 0's build chain unblocks almost
        # immediately, both in the scheduler's cost model and on HW.
        zin = stage_in.tile([P, NB, K], f32)
        lt2 = stage_in.tile([P, NB, K], f32)
        z_src = z_in.rearrange("p (a k) -> p a k", k=K)
        l_src = logits_t2_in.rearrange("p (a k) -> p a k", k=K)
        with tc.high_priority():
            for j in range(0, NB, 2):
                nc.gpsimd.dma_start(out=zin[:, j:j + 2, :], in_=z_src[:, j:j + 2, :])
                nc.gpsimd.dma_start(out=lt2[:, j:j + 2, :], in_=l_src[:, j:j + 2, :])

        with ExitStack() as psum_main:
            psum_T = psum_main.enter_context(tc.tile_pool(name="psum_T", bufs=1, space="PSUM"))
            psum_e = psum_main.enter_context(tc.tile_pool(name="psum_e", bufs=1, space="PSUM"))
            psum_s = psum_main.enter_context(tc.tile_pool(name="psum_s", bufs=1, space="PSUM"))
            T_ps = psum_T.tile([L, MH], f32)
            emb_a_ps = psum_e.tile([L, F], f32, tag="emb_a")
            emb_b_ps = psum_e.tile([L, F], f32, tag="emb_b")
            ss_ps = psum_s.tile([K, K], f32)

            # emb column-half blocks, each consumed twice (pairing a and b)
            et_tiles = []
            for j in range(JB):
                et = embp.tile([P, F], adj_dt)
                nc.gpsimd.dma_start(out=et[:, :], in_=emb_in[j * P:(j + 1) * P, :])
                et_tiles.append(et)

            l66 = []
            l66w = []
            at_tiles = {}
            for nb in range(NB):
                # 2MB adj DMA covering two row-blocks, alternating HWDGE rings
                if nb % 2 == 0:
                    at2 = adjp.tile([P, 2, MH], adj_dt)
                    dma_eng = nc.sync if (nb // 2) % 2 == 0 else nc.scalar
                    dma_eng.dma_start(
                        out=at2[:, :, :],
                        in_=adj_cols[nb * P:(nb + 2) * P, :].rearrange(
                            "(blk p) m -> p blk m", p=P),
                    )
                    at_tiles[nb] = at2

                lt = l66p.tile([P, L], f32)
                l66.append(lt)

                # --- s_sample = onehot(argmax(logits/t1 + gumbel)) + EPS ---
                zmax = col.tile([P, 1], f32, tag="zmax")
                nc.vector.reduce_max(zmax[:, :], zin[:, nb, :], axis=AX.X)
                tb = scr.tile([P, K], f32, tag="tb")
                nc.vector.scalar_tensor_tensor(
                    out=tb[:, :], in0=zin[:, nb, :], scalar=zmax[:, :], in1=iota_desc[:, :],
                    op0=OP.is_equal, op1=OP.mult,
                )
                tmax = col.tile([P, 1], f32, tag="tmax")
                nc.vector.reduce_max(tmax[:, :], tb[:, :], axis=AX.X)
                nc.vector.tensor_scalar(
                    out=lt[:, 0:K], in0=tb[:, :], scalar1=tmax[:, :], scalar2=float(EPS),
                    op0=OP.is_equal, op1=OP.add,
                )
                nc.gpsimd.dma_start(out=s_sample_o[nb * P:(nb + 1) * P, :], in_=lt[:, 0:K])

                # --- s_soft = softmax(logits/t2) ---
                ex = scr.tile([P, K], f32, tag="ex")
                esum = col.tile([P, 1], f32, tag="esum")
                nc.scalar.activation(out=ex[:, :], in_=lt2[:, nb, :], func=ACT.Exp,
                                     bias=zero_bias[:, :], accum_out=esum[:, :])
                rsum = col.tile([P, 1], f32, tag="rsum")
                nc.vector.reciprocal(rsum[:, :], esum[:, :])
                nc.vector.tensor_scalar_mul(lt[:, K:2 * K], ex[:, :], rsum[:, :])

                # --- q = sum_k s_soft^2 ; ones --- (all-DVE so lt has a
                # single producing engine, keeping matmul wait counts low)
                sq = scr.tile([P, K], f32, tag="sq")
                nc.vector.tensor_mul(sq[:, :], lt[:, K:2 * K], lt[:, K:2 * K])
                nc.vector.reduce_sum(lt[:, 2 * K:2 * K + 1], sq[:, :], axis=AX.X)
                nc.vector.memset(lt[:, 2 * K + 1:2 * K + 2], 1.0)

                # f32r-rounded copy of L_nb; all PE consumers read this one
                # DVE-produced tile. Kept live through stage 2.
                if USE_F32R:
                    ltw = l66rp.tile([P, L], f32r)
                    nc.vector.tensor_copy(ltw[:, :], lt[:, :])
                else:
                    ltw = lt
                l66w.append(ltw)

                # --- ss += s_soft_nb^T s_soft_nb ---
                nc.tensor.matmul(ss_ps[:, :], lhsT=ltw[:, K:2 * K], rhs=ltw[:, K:2 * K],
                                 start=(nb == 0), stop=(nb == NB - 1))

                # --- adj block: T66 += L_nb^T @ adj[nb rows, :] ---
                at = at_tiles[nb - nb % 2]
                blk = nb % 2
                for mb in range(MB):
                    nc.tensor.matmul(
                        T_ps[:, mb * 512:(mb + 1) * 512],
                        lhsT=ltw[:, :],
                        rhs=at[:, blk, mb * 512:(mb + 1) * 512],
                        start=(nb == 0), stop=(nb == NB - 1),
                    )

                # --- emb: each core streams only its emb row-half; both
                # pairings are computed (host picks a for even cores, b for
                # odd) and the two cores' picks sum to the full out_emb.
                if nb < JB:
                    nc.tensor.matmul(emb_a_ps[:, :], lhsT=ltw[:, :], rhs=et_tiles[nb][:, :],
                                     start=(nb == 0), stop=(nb == JB - 1))
                else:
                    nc.tensor.matmul(emb_b_ps[:, :], lhsT=ltw[:, :], rhs=et_tiles[nb - JB][:, :],
                                     start=(nb == JB), stop=(nb == NB - 1))

            # copy accumulators out of PSUM
            T_sb = t66p.tile([L, MH], f32)
            for mb in range(MB):
                nc.vector.tensor_copy(T_sb[:, mb * 512:(mb + 1) * 512],
                                      T_ps[:, mb * 512:(mb + 1) * 512])
            oea_sb = outp.tile([K, F], f32, tag="oea")
            nc.vector.tensor_copy(oea_sb[:, :], emb_a_ps[0:K, :])
            nc.sync.dma_start(out=out_emb_a_o[:, :], in_=oea_sb[:, :])
            oeb_sb = outp.tile([K, F], f32, tag="oeb")
            nc.vector.tensor_copy(oeb_sb[:, :], emb_b_ps[0:K, :])
            nc.sync.dma_start(out=out_emb_b_o[:, :], in_=oeb_sb[:, :])
            ss_sb = outp.tile([K, K], f32)
            nc.vector.tensor_copy(ss_sb[:, :], ss_ps[:, :])
            nc.sync.dma_start(out=ss_o[:, :], in_=ss_sb[:, :])

        # ---- stage 2: out2 = L_local^T @ T66^T over 16 local column blocks.
        # SPMD cores don't know their column half, so compute both pairings;
        # the host picks a (blocks 0:16) for even cores, b (16:32) for odd.
        with ExitStack() as psum_s2:
            psum_tr = psum_s2.enter_context(tc.tile_pool(name="psum_tr", bufs=4, space="PSUM"))
            psum_o2 = psum_s2.enter_context(tc.tile_pool(name="psum_o2", bufs=1, space="PSUM"))
            o2a_ps = psum_o2.tile([L, L], f32, tag="o2a")
            o2b_ps = psum_o2.tile([L, L], f32, tag="o2b")
            wdt = f32r if USE_F32R else f32
            for j in range(JB):
                tr_ps = psum_tr.tile([P, L], f32)
                nc.tensor.transpose(tr_ps[:, :], T_sb[:, j * P:(j + 1) * P], identity66[:, :])
                tr_sb = trp.tile([P, L], wdt)
                nc.vector.tensor_copy(tr_sb[:, :], tr_ps[:, :])
                nc.tensor.matmul(o2a_ps[:, :], lhsT=l66w[j][:, :], rhs=tr_sb[:, :],
                                 start=(j == 0), stop=(j == JB - 1))
                nc.tensor.matmul(o2b_ps[:, :], lhsT=l66w[JB + j][:, :], rhs=tr_sb[:, :],
                                 start=(j == 0), stop=(j == JB - 1))
            o2a_sb = outp.tile([L, L], f32, tag="o2a_sb")
            nc.vector.tensor_copy(o2a_sb[:, :], o2a_ps[:, :])
            nc.sync.dma_start(out=out2a_o[:, :], in_=o2a_sb[:, :])
            o2b_sb = outp.tile([L, L], f32, tag="o2b_sb")
            nc.vector.tensor_copy(o2b_sb[:, :], o2b_ps[:, :])
            nc.sync.dma_start(out=out2b_o[:, :], in_=o2b_sb[:, :])

    nc.compile()
    return nc


def get_program():
    if "nc" not in _cache:
        _cache["nc"] = _build_program()
    return _cache["nc"]


def run_cores(inputs, **run_kwargs):
    """Build in_maps, run on 8 cores, return per-core result dicts."""
    from concourse.bass_utils import run_bass_kernel_spmd

    emb = np.asarray(inputs["emb"], dtype=np.float32)
    adj = np.asarray(inputs["adj"], dtype=np.float32)
    logits = np.asarray(inputs["logits_param"], dtype=np.float32)
    gumbel = np.asarray(inputs["gumbel_noise"], dtype=np.float32)

    logits_t1 = (logits / np.float32(T1)).astype(np.float32)
    logits_t2 = (logits / np.float32(T2)).astype(np.float32)

    def to_pnk(x):
        # [N, K] row-major -> [P, NB*K] where element (p, nb*K+k) = x[nb*P+p, k]
        return np.ascontiguousarray(
            x.reshape(NB, P, K).transpose(1, 0, 2).reshape(P, NB * K)
        )

    lt2_pnk = to_pnk(logits_t2)

    in_maps = []
    for c in range(8):
        b, h = c // 2, c % 2
        in_maps.append({
            "adj_cols": np.ascontiguousarray(adj[b, :, h * MH:(h + 1) * MH]),
            "emb_in": np.ascontiguousarray(emb[b, h * MH:(h + 1) * MH]),
            "z_in": to_pnk((logits_t1 + gumbel[b]).astype(np.float32)),
            "logits_t2": lt2_pnk,
        })

    nc = get_program()
    return run_bass_kernel_spmd(nc, in_maps, core_ids=list(range(8)), **run_kwargs)


def finalize(results):
    """Host-side unshard + tiny [K,K]-level finalization (fp32)."""
    out_emb = np.zeros((B, K, F), dtype=np.float32)
    out_adj = np.zeros((B, K, K), dtype=np.float32)
    s_sample = np.zeros((B, N, K), dtype=np.float32)
    num = np.zeros(B, dtype=np.float32)
    den = np.zeros(B, dtype=np.float32)

    for b in range(B):
        re, ro = results[2 * b], results[2 * b + 1]
        out2 = re["out2a_o"] + ro["out2b_o"]
        out_emb[b] = re["out_emb_a_o"] + ro["out_emb_b_o"]
        s_sample[b] = re["s_sample_o"]
        G = out2[0:K, 0:K].T
        num[b] = np.trace(out2[K:2 * K, K:2 * K])
        den[b] = out2[2 * K + 1, 2 * K]
        A = G * (np.float32(1.0) - np.eye(K, dtype=np.float32))
        d = np.sqrt(A.sum(axis=-1, keepdims=True)) + np.float32(EPS)
        out_adj[b] = A / d / d.T

    mincut_loss = np.float32(-np.mean(num / den))

    ss = results[0]["ss_o"]
    ss_n = ss / np.linalg.norm(ss)
    ortho_loss = np.float32(
        np.linalg.norm(ss_n - np.eye(K, dtype=np.float32) / np.float32(np.sqrt(K)))
    )
    return out_emb, out_adj, s_sample, mincut_loss, ortho_loss


def kernel(**inputs):
    res = run_cores(inputs)
    return finalize(res.results)


# revision 51
# speedup vs baseline: 1.0039x; 1.0039x over previous
"""MinCutPool forward on 8 trn2 NeuronCores.

Shapes: emb [4,4096,128], adj [4,4096,4096], logits_param [4096,32],
gumbel_noise [4,4096,32].

Sharding: core c handles batch b=c//2 and adj column-half h=c%2.

Math (per batch b, with L = [s_sample | s_soft | q | ones]  [N, 66]):
    T66 = L^T @ adj[:, cols]            # [66, 2048]  contraction over rows n
      rows 0:32  = s_sample^T A  (T_sa)
      rows 32:64 = s_soft^T A    (T_ss)
      row  64    = q^T A = r     (q[n] = sum_k s_soft[n,k]^2)
      row  65    = ones^T A      (unused)
    out2 = L_local^T @ T66^T            # [66, 66]  contraction over local cols m
      out2[0:32,0:32][l,k]   = G[k,l] partial   (G = s_sample^T A s_sample)
      trace(out2[32:64,32:64]) = mincut_num partial
      out2[65,64]            = mincut_den partial
    out_emb = s_sample^T @ emb          # rows 0:32 of L^T @ emb
    ss = s_soft^T s_soft                # for ortho_loss

Host finalizes the tiny [K,K]-level math (pair-sums, normalization, losses).
"""

import os
import sys

sys.path.insert(0, "/opt/trn_rl_repo")

import numpy as np

B, N, F, K = 4, 4096, 128, 32
P = 128                  # partitions
NB = N // P              # 32 row blocks
MH = N // 2              # 2048 local adj columns per core
MB = MH // 512           # 4 T-matmul output column blocks
JB = MH // P             # 16 stage-2 blocks
L = 2 * K + 2            # 66 fused weight columns
EPS = 1e-8
T1 = max(1.0 * 0.99995, 0.05)
T2 = max(T1 * 0.99995, 0.05)

USE_F32R = os.environ.get("KERNEL_F32R", "1") == "1"

_cache = {}


def _build_program():
    from contextlib import ExitStack

    import concourse.bass as bass
    import concourse.bacc as bacc
    import concourse.tile as tile
    from concourse import mybir
    from concourse.masks import make_identity

    f32 = mybir.dt.float32
    f32r = mybir.dt.float32r
    AX = mybir.AxisListType
    OP = mybir.AluOpType
    ACT = mybir.ActivationFunctionType

    nc = bacc.Bacc("TRN2", target_bir_lowering=False)

    adj_dt = f32r if USE_F32R else f32
    adj_cols = nc.dram_tensor("adj_cols", [N, MH], adj_dt, kind="ExternalInput")
    emb_in = nc.dram_tensor("emb_in", [MH, F], adj_dt, kind="ExternalInput")
    # host pre-permutes these to [p, nb, k] so the staging DMA is contiguous
    z_in = nc.dram_tensor("z_in", [P, NB * K], f32, kind="ExternalInput")
    logits_t2_in = nc.dram_tensor("logits_t2", [P, NB * K], f32, kind="ExternalInput")

    s_sample_o = nc.dram_tensor("s_sample_o", [N, K], f32, kind="ExternalOutput")
    out_emb_a_o = nc.dram_tensor("out_emb_a_o", [K, F], f32, kind="ExternalOutput")
    out_emb_b_o = nc.dram_tensor("out_emb_b_o", [K, F], f32, kind="ExternalOutput")
    out2a_o = nc.dram_tensor("out2a_o", [L, L], f32, kind="ExternalOutput")
    out2b_o = nc.dram_tensor("out2b_o", [L, L], f32, kind="ExternalOutput")
    ss_o = nc.dram_tensor("ss_o", [K, K], f32, kind="ExternalOutput")

    def mmdt(ap):
        return ap.bitcast(f32r) if USE_F32R else ap

    with tile.TileContext(nc) as tc, ExitStack() as ctx:
        consts = ctx.enter_context(tc.tile_pool(name="consts", bufs=1))
        stage_in = ctx.enter_context(tc.tile_pool(name="stage_in", bufs=1))
        l66p = ctx.enter_context(tc.tile_pool(name="l66", bufs=NB))
        scr = ctx.enter_context(tc.tile_pool(name="scr", bufs=8))
        col = ctx.enter_context(tc.tile_pool(name="col", bufs=12))
        adjp = ctx.enter_context(tc.tile_pool(name="adjp", bufs=4))
        l66rp = ctx.enter_context(tc.tile_pool(name="l66r", bufs=NB))
        embp = ctx.enter_context(tc.tile_pool(name="embp", bufs=JB))
        t66p = ctx.enter_context(tc.tile_pool(name="t66sb", bufs=1))
        trp = ctx.enter_context(tc.tile_pool(name="trsb", bufs=3))
        outp = ctx.enter_context(tc.tile_pool(name="outp", bufs=1))

        # iota_desc[p, k] = K - k  (32..1), same on every partition: used to
        # pick the FIRST argmax on ties, matching jnp.argmax. Generated on
        # gpsimd, then bounced through a DVE copy so the hot DVE consumer's
        # producer is same-engine (keeps per-instruction wait counts low).
        iota_g = consts.tile([P, K], f32, tag="iota_g")
        nc.gpsimd.iota(
            iota_g[:, :],
            pattern=[[-1, K]],
            base=K,
            channel_multiplier=0,
            allow_small_or_imprecise_dtypes=True,
        )
        iota_desc = consts.tile([P, K], f32, tag="iota_desc")
        nc.vector.tensor_copy(iota_desc[:, :], iota_g[:, :])
        # ACT-produced zero bias (scale=0 copy) so Exp's bias needs no
        # cross-engine wait.
        zero_bias = consts.tile([P, 1], f32, tag="zero_bias")
        nc.scalar.mul(zero_bias[:, :], zero_bias[:, :], 0.0)
        ident_g = consts.tile([L, L], f32, tag="ident_g")
        make_identity(nc, ident_g[:, :])
        identity66 = consts.tile([L, L], f32, tag="identity66")
        nc.vector.tensor_copy(identity66[:, :], ident_g[:, :])

        # z = logits/t1 + gumbel (host-computed) and logits/t2, already
        # host-permuted to [p, nb, k]. One on each HWDGE ring, issued before
        # any adj DMA: ring FIFO then guarantees they transfer first, so the
        # build chain (which gates the first matmul) starts ASAP.
        with tc.high_priority():
            zin = stage_in.tile([P, NB, K], f32)
            nc.sync.dma_start(out=zin[:, :, :], in_=z_in.rearrange("p (a k) -> p a k", k=K))
            lt2 = stage_in.tile([P, NB, K], f32)
            nc.scalar.dma_start(out=lt2[:, :, :], in_=logits_t2_in.rearrange("p (a k) -> p a k", k=K))

        with ExitStack() as psum_main:
            psum_T = psum_main.enter_context(tc.tile_pool(name="psum_T", bufs=1, space="PSUM"))
            psum_e = psum_main.enter_context(tc.tile_pool(name="psum_e", bufs=1, space="PSUM"))
            psum_s = psum_main.enter_context(tc.tile_pool(name="psum_s", bufs=1, space="PSUM"))
            T_ps = psum_T.tile([L, MH], f32)
            emb_a_ps = psum_e.tile([L, F], f32, tag="emb_a")
            emb_b_ps = psum_e.tile([L, F], f32, tag="emb_b")
            ss_ps = psum_s.tile([K, K], f32)

            # emb column-half blocks, each consumed twice (pairing a and b)
            et_tiles = []
            for j in range(JB):
                et = embp.tile([P, F], adj_dt)
                nc.scalar.dma_start(out=et[:, :], in_=emb_in[j * P:(j + 1) * P, :])
                et_tiles.append(et)

            l66 = []
            l66w = []
            at_tiles = {}
            for nb in range(NB):
                # 2MB adj DMA covering two row-blocks, alternating HWDGE rings
                if nb % 2 == 0:
                    # Alternate sync (HWDGE) / gpsimd (SWDGE): two parallel
                    # DMA paths, and neither occupies the ACT sequencer,
                    # which must stay free for the EXP chain.
                    at2 = adjp.tile([P, 2, MH], adj_dt)
                    dma_eng = nc.sync if (nb // 2) % 2 == 0 else nc.gpsimd
                    dma_eng.dma_start(
                        out=at2[:, :, :],
                        in_=adj_cols[nb * P:(nb + 2) * P, :].rearrange(
                            "(blk p) m -> p blk m", p=P),
                    )
                    at_tiles[nb] = at2

                lt = l66p.tile([P, L], f32)
                l66.append(lt)

                # --- s_sample = onehot(argmax(logits/t1 + gumbel)) + EPS ---
                zmax = col.tile([P, 1], f32, tag="zmax")
                nc.vector.reduce_max(zmax[:, :], zin[:, nb, :], axis=AX.X)
                tb = scr.tile([P, K], f32, tag="tb")
                nc.vector.scalar_tensor_tensor(
                    out=tb[:, :], in0=zin[:, nb, :], scalar=zmax[:, :], in1=iota_desc[:, :],
                    op0=OP.is_equal, op1=OP.mult,
                )
                tmax = col.tile([P, 1], f32, tag="tmax")
                nc.vector.reduce_max(tmax[:, :], tb[:, :], axis=AX.X)
                nc.vector.tensor_scalar(
                    out=lt[:, 0:K], in0=tb[:, :], scalar1=tmax[:, :], scalar2=float(EPS),
                    op0=OP.is_equal, op1=OP.add,
                )
                nc.scalar.dma_start(out=s_sample_o[nb * P:(nb + 1) * P, :], in_=lt[:, 0:K])

                # --- s_soft = softmax(logits/t2) ---
                ex = scr.tile([P, K], f32, tag="ex")
                esum = col.tile([P, 1], f32, tag="esum")
                nc.scalar.activation(out=ex[:, :], in_=lt2[:, nb, :], func=ACT.Exp,
                                     bias=zero_bias[:, :], accum_out=esum[:, :])
                rsum = col.tile([P, 1], f32, tag="rsum")
                nc.vector.reciprocal(rsum[:, :], esum[:, :])
                nc.vector.tensor_scalar_mul(lt[:, K:2 * K], ex[:, :], rsum[:, :])

                # --- q = sum_k s_soft^2 ; ones --- (all-DVE so lt has a
                # single producing engine, keeping matmul wait counts low)
                sq = scr.tile([P, K], f32, tag="sq")
                nc.vector.tensor_mul(sq[:, :], lt[:, K:2 * K], lt[:, K:2 * K])
                nc.vector.reduce_sum(lt[:, 2 * K:2 * K + 1], sq[:, :], axis=AX.X)
                nc.vector.memset(lt[:, 2 * K + 1:2 * K + 2], 1.0)

                # f32r-rounded copy of L_nb; all PE consumers read this one
                # DVE-produced tile. Kept live through stage 2.
                if USE_F32R:
                    ltw = l66rp.tile([P, L], f32r)
                    nc.vector.tensor_copy(ltw[:, :], lt[:, :])
                else:
                    ltw = lt
                l66w.append(ltw)

                # --- ss += s_soft_nb^T s_soft_nb ---
                nc.tensor.matmul(ss_ps[:, :], lhsT=ltw[:, K:2 * K], rhs=ltw[:, K:2 * K],
                                 start=(nb == 0), stop=(nb == NB - 1))

                # --- adj block: T66 += L_nb^T @ adj[nb rows, :] ---
                at = at_tiles[nb - nb % 2]
                blk = nb % 2
                for mb in range(MB):
                    nc.tensor.matmul(
                        T_ps[:, mb * 512:(mb + 1) * 512],
                        lhsT=ltw[:, :],
                        rhs=at[:, blk, mb * 512:(mb + 1) * 512],
                        start=(nb == 0), stop=(nb == NB - 1),
                    )

                # --- emb: each core streams only its emb row-half; both
                # pairings are computed (host picks a for even cores, b for
                # odd) and the two cores' picks sum to the full out_emb.
                if nb < JB:
                    nc.tensor.matmul(emb_a_ps[:, :], lhsT=ltw[:, :], rhs=et_tiles[nb][:, :],
                                     start=(nb == 0), stop=(nb == JB - 1))
                else:
                    nc.tensor.matmul(emb_b_ps[:, :], lhsT=ltw[:, :], rhs=et_tiles[nb - JB][:, :],
                                     start=(nb == JB), stop=(nb == NB - 1))

            # copy accumulators out of PSUM
            T_sb = t66p.tile([L, MH], f32)
            for mb in range(MB):
                nc.vector.tensor_copy(T_sb[:, mb * 512:(mb + 1) * 512],
                                      T_ps[:, mb * 512:(mb + 1) * 512])
            oea_sb = outp.tile([K, F], f32, tag="oea")
            nc.vector.tensor_copy(oea_sb[:, :], emb_a_ps[0:K, :])
            nc.sync.dma_start(out=out_emb_a_o[:, :], in_=oea_sb[:, :])
            oeb_sb = outp.tile([K, F], f32, tag="oeb")
            nc.vector.tensor_copy(oeb_sb[:, :], emb_b_ps[0:K, :])
            nc.sync.dma_start(out=out_emb_b_o[:, :], in_=oeb_sb[:, :])
            ss_sb = outp.tile([K, K], f32)
            nc.vector.tensor_copy(ss_sb[:, :], ss_ps[:, :])
            nc.sync.dma_start(out=ss_o[:, :], in_=ss_sb[:, :])

        # ---- stage 2: out2 = L_local^T @ T66^T over 16 local column blocks.
        # SPMD cores don't know their column half, so compute both pairings;
        # the host picks a (blocks 0:16) for even cores, b (16:32) for odd.
        with ExitStack() as psum_s2:
            psum_tr = psum_s2.enter_context(tc.tile_pool(name="psum_tr", bufs=4, space="PSUM"))
            psum_o2 = psum_s2.enter_context(tc.tile_pool(name="psum_o2", bufs=1, space="PSUM"))
            o2a_ps = psum_o2.tile([L, L], f32, tag="o2a")
            o2b_ps = psum_o2.tile([L, L], f32, tag="o2b")
            wdt = f32r if USE_F32R else f32
            for j in range(JB):
                tr_ps = psum_tr.tile([P, L], f32)
                nc.tensor.transpose(tr_ps[:, :], T_sb[:, j * P:(j + 1) * P], identity66[:, :])
                tr_sb = trp.tile([P, L], wdt)
                nc.vector.tensor_copy(tr_sb[:, :], tr_ps[:, :])
                nc.tensor.matmul(o2a_ps[:, :], lhsT=l66w[j][:, :], rhs=tr_sb[:, :],
                                 start=(j == 0), stop=(j == JB - 1))
                nc.tensor.matmul(o2b_ps[:, :], lhsT=l66w[JB + j][:, :], rhs=tr_sb[:, :],
                                 start=(j == 0), stop=(j == JB - 1))
            o2a_sb = outp.tile([L, L], f32, tag="o2a_sb")
            nc.vector.tensor_copy(o2a_sb[:, :], o2a_ps[:, :])
            nc.sync.dma_start(out=out2a_o[:, :], in_=o2a_sb[:, :])
            o2b_sb = outp.tile([L, L], f32, tag="o2b_sb")
            nc.vector.tensor_copy(o2b_sb[:, :], o2b_ps[:, :])
            nc.sync.dma_start(out=out2b_o[:, :], in_=o2b_sb[:, :])

    nc.compile()
    return nc


def get_program():
    if "nc" not in _cache:
        _cache["nc"] = _build_program()
    return _cache["nc"]


def run_cores(inputs, **run_kwargs):
    """Build in_maps, run on 8 cores, return per-core result dicts."""
    from concourse.bass_utils import run_bass_kernel_spmd

    emb = np.asarray(inputs["emb"], dtype=np.float32)
    adj = np.asarray(inputs["adj"], dtype=np.float32)
    logits = np.asarray(inputs["logits_param"], dtype=np.float32)
    gumbel = np.asarray(inputs["gumbel_noise"], dtype=np.float32)

    logits_t1 = (logits / np.float32(T1)).astype(np.float32)
    logits_t2 = (logits / np.float32(T2)).astype(np.float32)

    def to_pnk(x):
        # [N, K] row-major -> [P, NB*K] where element (p, nb*K+k) = x[nb*P+p, k]
        return np.ascontiguousarray(
            x.reshape(NB, P, K).transpose(1, 0, 2).reshape(P, NB * K)
        )

    lt2_pnk = to_pnk(logits_t2)

    in_maps = []
    for c in range(8):
        b, h = c // 2, c % 2
        in_maps.append({
            "adj_cols": np.ascontiguousarray(adj[b, :, h * MH:(h + 1) * MH]),
            "emb_in": np.ascontiguousarray(emb[b, h * MH:(h + 1) * MH]),
            "z_in": to_pnk((logits_t1 + gumbel[b]).astype(np.float32)),
            "logits_t2": lt2_pnk,
        })

    nc = get_program()
    return run_bass_kernel_spmd(nc, in_maps, core_ids=list(range(8)), **run_kwargs)


def finalize(results):
    """Host-side unshard + tiny [K,K]-level finalization (fp32)."""
    out_emb = np.zeros((B, K, F), dtype=np.float32)
    out_adj = np.zeros((B, K, K), dtype=np.float32)
    s_sample = np.zeros((B, N, K), dtype=np.float32)
    num = np.zeros(B, dtype=np.float32)
    den = np.zeros(B, dtype=np.float32)

    for b in range(B):
        re, ro = results[2 * b], results[2 * b + 1]
        out2 = re["out2a_o"] + ro["out2b_o"]
        out_emb[b] = re["out_emb_a_o"] + ro["out_emb_b_o"]
        s_sample[b] = re["s_sample_o"]
        G = out2[0:K, 0:K].T
        num[b] = np.trace(out2[K:2 * K, K:2 * K])
        den[b] = out2[2 * K + 1, 2 * K]
        A = G * (np.float32(1.0) - np.eye(K, dtype=np.float32))
        d = np.sqrt(A.sum(axis=-1, keepdims=True)) + np.float32(EPS)
        out_adj[b] = A / d / d.T

    mincut_loss = np.float32(-np.mean(num / den))

    ss = results[0]["ss_o"]
    ss_n = ss / np.linalg.norm(ss)
    ortho_loss = np.float32(
        np.linalg.norm(ss_n - np.eye(K, dtype=np.float32) / np.float32(np.sqrt(K)))
    )
    return out_emb, out_adj, s_sample, mincut_loss, ortho_loss


def kernel(**inputs):
    res = run_cores(inputs)
    return finalize(res.results)


# revision 55
# speedup vs baseline: 1.1797x; 1.1750x over previous
"""MinCutPool forward on 8 trn2 NeuronCores.

Shapes: emb [4,4096,128], adj [4,4096,4096], logits_param [4096,32],
gumbel_noise [4,4096,32].

Sharding: core c handles batch b=c//2 and adj column-half h=c%2.

Math (per batch b, with L = [s_sample | s_soft | q | ones]  [N, 66]):
    T66 = L^T @ adj[:, cols]            # [66, 2048]  contraction over rows n
      rows 0:32  = s_sample^T A  (T_sa)
      rows 32:64 = s_soft^T A    (T_ss)
      row  64    = q^T A = r     (q[n] = sum_k s_soft[n,k]^2)
      row  65    = ones^T A      (unused)
    out2 = L_local^T @ T66^T            # [66, 66]  contraction over local cols m
      out2[0:32,0:32][l,k]   = G[k,l] partial   (G = s_sample^T A s_sample)
      trace(out2[32:64,32:64]) = mincut_num partial
      out2[65,64]            = mincut_den partial
    out_emb = s_sample^T @ emb          # rows 0:32 of L^T @ emb
    ss = s_soft^T s_soft                # for ortho_loss

Host finalizes the tiny [K,K]-level math (pair-sums, normalization, losses).
"""

import os
import sys

sys.path.insert(0, "/opt/trn_rl_repo")

import numpy as np

B, N, F, K = 4, 4096, 128, 32
P = 128                  # partitions
NB = N // P              # 32 row blocks
MH = N // 2              # 2048 local adj columns per core
MB = MH // 512           # 4 T-matmul output column blocks
JB = MH // P             # 16 stage-2 blocks
L = 2 * K + 2            # 66 fused weight columns
EPS = 1e-8
T1 = max(1.0 * 0.99995, 0.05)
T2 = max(T1 * 0.99995, 0.05)

USE_F32R = os.environ.get("KERNEL_F32R", "1") == "1"

_cache = {}


def _build_program():
    from contextlib import ExitStack

    import concourse.bass as bass
    import concourse.bacc as bacc
    import concourse.tile as tile
    from concourse import mybir
    from concourse.masks import make_identity

    f32 = mybir.dt.float32
    f32r = mybir.dt.float32r
    AX = mybir.AxisListType
    OP = mybir.AluOpType
    ACT = mybir.ActivationFunctionType

    nc = bacc.Bacc("TRN2", target_bir_lowering=False)

    adj_dt = f32r if USE_F32R else f32
    adj_cols = nc.dram_tensor("adj_cols", [N, MH], adj_dt, kind="ExternalInput")
    emb_in = nc.dram_tensor("emb_in", [MH, F], adj_dt, kind="ExternalInput")
    # host pre-permutes these to [p, nb, k] so the staging DMA is contiguous
    z_in = nc.dram_tensor("z_in", [P, NB * K], f32, kind="ExternalInput")
    logits_t2_in = nc.dram_tensor("logits_t2", [P, NB * K], f32, kind="ExternalInput")

    s_sample_o = nc.dram_tensor("s_sample_o", [N, K], f32, kind="ExternalOutput")
    out_emb_a_o = nc.dram_tensor("out_emb_a_o", [K, F], f32, kind="ExternalOutput")
    out_emb_b_o = nc.dram_tensor("out_emb_b_o", [K, F], f32, kind="ExternalOutput")
    out2a_o = nc.dram_tensor("out2a_o", [L, L], f32, kind="ExternalOutput")
    out2b_o = nc.dram_tensor("out2b_o", [L, L], f32, kind="ExternalOutput")
    ss_o = nc.dram_tensor("ss_o", [K, K], f32, kind="ExternalOutput")

    def mmdt(ap):
        return ap.bitcast(f32r) if USE_F32R else ap

    with tile.TileContext(nc) as tc, ExitStack() as ctx:
        consts = ctx.enter_context(tc.tile_pool(name="consts", bufs=1))
        stage_in = ctx.enter_context(tc.tile_pool(name="stage_in", bufs=1))
        l66p = ctx.enter_context(tc.tile_pool(name="l66", bufs=NB))
        scr = ctx.enter_context(tc.tile_pool(name="scr", bufs=8))
        col = ctx.enter_context(tc.tile_pool(name="col", bufs=12))
        adjp = ctx.enter_context(tc.tile_pool(name="adjp", bufs=4))
        l66rp = ctx.enter_context(tc.tile_pool(name="l66r", bufs=NB))
        embp = ctx.enter_context(tc.tile_pool(name="embp", bufs=JB))
        t66p = ctx.enter_context(tc.tile_pool(name="t66sb", bufs=1))
        trp = ctx.enter_context(tc.tile_pool(name="trsb", bufs=3))
        outp = ctx.enter_context(tc.tile_pool(name="outp", bufs=1))

        # iota_desc[p, k] = K - k  (32..1), same on every partition: used to
        # pick the FIRST argmax on ties, matching jnp.argmax. Generated on
        # gpsimd, then bounced through a DVE copy so the hot DVE consumer's
        # producer is same-engine (keeps per-instruction wait counts low).
        iota_g = consts.tile([P, K], f32, tag="iota_g")
        nc.gpsimd.iota(
            iota_g[:, :],
            pattern=[[-1, K]],
            base=K,
            channel_multiplier=0,
            allow_small_or_imprecise_dtypes=True,
        )
        iota_desc = consts.tile([P, K], f32, tag="iota_desc")
        nc.vector.tensor_copy(iota_desc[:, :], iota_g[:, :])
        # ACT-produced zero bias (scale=0 copy) so Exp's bias needs no
        # cross-engine wait.
        zero_bias = consts.tile([P, 1], f32, tag="zero_bias")
        nc.scalar.mul(zero_bias[:, :], zero_bias[:, :], 0.0)
        ident_g = consts.tile([L, L], f32, tag="ident_g")
        make_identity(nc, ident_g[:, :])
        identity66 = consts.tile([L, L], f32, tag="identity66")
        nc.vector.tensor_copy(identity66[:, :], ident_g[:, :])

        # z = logits/t1 + gumbel (host-computed) and logits/t2, already
        # host-permuted to [p, nb, k]. One on each HWDGE ring, issued before
        # any adj DMA: ring FIFO then guarantees they transfer first, so the
        # build chain (which gates the first matmul) starts ASAP.
        with tc.high_priority():
            zin = stage_in.tile([P, NB, K], f32)
            nc.sync.dma_start(out=zin[:, :, :], in_=z_in.rearrange("p (a k) -> p a k", k=K))
            lt2 = stage_in.tile([P, NB, K], f32)
            nc.scalar.dma_start(out=lt2[:, :, :], in_=logits_t2_in.rearrange("p (a k) -> p a k", k=K))

        with ExitStack() as psum_main:
            psum_T = psum_main.enter_context(tc.tile_pool(name="psum_T", bufs=1, space="PSUM"))
            psum_e = psum_main.enter_context(tc.tile_pool(name="psum_e", bufs=1, space="PSUM"))
            psum_s = psum_main.enter_context(tc.tile_pool(name="psum_s", bufs=1, space="PSUM"))
            T_ps = psum_T.tile([L, MH], f32)
            emb_a_ps = psum_e.tile([L, F], f32, tag="emb_a")
            emb_b_ps = psum_e.tile([L, F], f32, tag="emb_b")
            ss_ps = psum_s.tile([K, K], f32)

            # emb column-half blocks, each consumed twice (pairing a and b)
            et_tiles = []
            for j in range(JB):
                et = embp.tile([P, F], adj_dt)
                nc.gpsimd.dma_start(out=et[:, :], in_=emb_in[j * P:(j + 1) * P, :])
                et_tiles.append(et)

            l66 = []
            l66w = []
            at_tiles = {}
            for nb in range(NB):
                # 2MB adj DMA covering two row-blocks, alternating HWDGE rings
                if nb % 2 == 0:
                    at2 = adjp.tile([P, 2, MH], adj_dt)
                    dma_eng = nc.sync if (nb // 2) % 2 == 0 else nc.scalar
                    dma_eng.dma_start(
                        out=at2[:, :, :],
                        in_=adj_cols[nb * P:(nb + 2) * P, :].rearrange(
                            "(blk p) m -> p blk m", p=P),
                    )
                    at_tiles[nb] = at2

                lt = l66p.tile([P, L], f32)
                l66.append(lt)

                # --- s_sample = onehot(argmax(logits/t1 + gumbel)) + EPS ---
                zmax = col.tile([P, 1], f32, tag="zmax")
                nc.vector.reduce_max(zmax[:, :], zin[:, nb, :], axis=AX.X)
                tb = scr.tile([P, K], f32, tag="tb")
                nc.vector.scalar_tensor_tensor(
                    out=tb[:, :], in0=zin[:, nb, :], scalar=zmax[:, :], in1=iota_desc[:, :],
                    op0=OP.is_equal, op1=OP.mult,
                )
                tmax = col.tile([P, 1], f32, tag="tmax")
                nc.vector.reduce_max(tmax[:, :], tb[:, :], axis=AX.X)
                nc.vector.tensor_scalar(
                    out=lt[:, 0:K], in0=tb[:, :], scalar1=tmax[:, :], scalar2=float(EPS),
                    op0=OP.is_equal, op1=OP.add,
                )
                nc.gpsimd.dma_start(out=s_sample_o[nb * P:(nb + 1) * P, :], in_=lt[:, 0:K])

                # --- s_soft = softmax(logits/t2) --- high-priority so the
                # scheduler interleaves this chain (which gates the weights
                # CAST and hence every matmul of this block) ahead of other
                # blocks' one-hot waves.
                with tc.high_priority():
                    ex = scr.tile([P, K], f32, tag="ex")
                    esum = col.tile([P, 1], f32, tag="esum")
                    nc.scalar.activation(out=ex[:, :], in_=lt2[:, nb, :], func=ACT.Exp,
                                         bias=zero_bias[:, :], accum_out=esum[:, :])
                    rsum = col.tile([P, 1], f32, tag="rsum")
                    nc.vector.reciprocal(rsum[:, :], esum[:, :])
                    nc.vector.tensor_scalar_mul(lt[:, K:2 * K], ex[:, :], rsum[:, :])

                    # --- q = sum_k s_soft^2 ; ones --- (all-DVE so lt has a
                    # single producing engine, keeping matmul wait counts low)
                    sq = scr.tile([P, K], f32, tag="sq")
                    nc.vector.tensor_mul(sq[:, :], lt[:, K:2 * K], lt[:, K:2 * K])
                    nc.vector.reduce_sum(lt[:, 2 * K:2 * K + 1], sq[:, :], axis=AX.X)
                    nc.vector.memset(lt[:, 2 * K + 1:2 * K + 2], 1.0)

                    # f32r-rounded copy of L_nb; all PE consumers read this
                    # one DVE-produced tile. Kept live through stage 2.
                    if USE_F32R:
                        ltw = l66rp.tile([P, L], f32r)
                        nc.vector.tensor_copy(ltw[:, :], lt[:, :])
                    else:
                        ltw = lt
                l66w.append(ltw)

                # --- ss += s_soft_nb^T s_soft_nb ---
                nc.tensor.matmul(ss_ps[:, :], lhsT=ltw[:, K:2 * K], rhs=ltw[:, K:2 * K],
                                 start=(nb == 0), stop=(nb == NB - 1))

                # --- adj block: T66 += L_nb^T @ adj[nb rows, :] ---
                at = at_tiles[nb - nb % 2]
                blk = nb % 2
                for mb in range(MB):
                    nc.tensor.matmul(
                        T_ps[:, mb * 512:(mb + 1) * 512],
                        lhsT=ltw[:, :],
                        rhs=at[:, blk, mb * 512:(mb + 1) * 512],
                        start=(nb == 0), stop=(nb == NB - 1),
                    )

                # --- emb: each core streams only its emb row-half; both
                # pairings are computed (host picks a for even cores, b for
                # odd) and the two cores' picks sum to the full out_emb.
                if nb < JB:
                    nc.tensor.matmul(emb_a_ps[:, :], lhsT=ltw[:, :], rhs=et_tiles[nb][:, :],
                                     start=(nb == 0), stop=(nb == JB - 1))
                else:
                    nc.tensor.matmul(emb_b_ps[:, :], lhsT=ltw[:, :], rhs=et_tiles[nb - JB][:, :],
                                     start=(nb == JB), stop=(nb == NB - 1))

            # copy accumulators out of PSUM
            T_sb = t66p.tile([L, MH], f32)
            for mb in range(MB):
                nc.vector.tensor_copy(T_sb[:, mb * 512:(mb + 1) * 512],
                                      T_ps[:, mb * 512:(mb + 1) * 512])
            oea_sb = outp.tile([K, F], f32, tag="oea")
            nc.vector.tensor_copy(oea_sb[:, :], emb_a_ps[0:K, :])
            nc.sync.dma_start(out=out_emb_a_o[:, :], in_=oea_sb[:, :])
            oeb_sb = outp.tile([K, F], f32, tag="oeb")
            nc.vector.tensor_copy(oeb_sb[:, :], emb_b_ps[0:K, :])
            nc.sync.dma_start(out=out_emb_b_o[:, :], in_=oeb_sb[:, :])
            ss_sb = outp.tile([K, K], f32)
            nc.vector.tensor_copy(ss_sb[:, :], ss_ps[:, :])
            nc.sync.dma_start(out=ss_o[:, :], in_=ss_sb[:, :])

        # ---- stage 2: out2 = L_local^T @ T66^T over 16 local column blocks.
        # SPMD cores don't know their column half, so compute both pairings;
        # the host picks a (blocks 0:16) for even cores, b (16:32) for odd.
        with ExitStack() as psum_s2:
            psum_tr = psum_s2.enter_context(tc.tile_pool(name="psum_tr", bufs=4, space="PSUM"))
            psum_o2 = psum_s2.enter_context(tc.tile_pool(name="psum_o2", bufs=1, space="PSUM"))
            o2a_ps = psum_o2.tile([L, L], f32, tag="o2a")
            o2b_ps = psum_o2.tile([L, L], f32, tag="o2b")
            wdt = f32r if USE_F32R else f32
            for j in range(JB):
                tr_ps = psum_tr.tile([P, L], f32)
                nc.tensor.transpose(tr_ps[:, :], T_sb[:, j * P:(j + 1) * P], identity66[:, :])
                tr_sb = trp.tile([P, L], wdt)
                nc.vector.tensor_copy(tr_sb[:, :], tr_ps[:, :])
                nc.tensor.matmul(o2a_ps[:, :], lhsT=l66w[j][:, :], rhs=tr_sb[:, :],
                                 start=(j == 0), stop=(j == JB - 1))
                nc.tensor.matmul(o2b_ps[:, :], lhsT=l66w[JB + j][:, :], rhs=tr_sb[:, :],
                                 start=(j == 0), stop=(j == JB - 1))
            o2a_sb = outp.tile([L, L], f32, tag="o2a_sb")
            nc.vector.tensor_copy(o2a_sb[:, :], o2a_ps[:, :])
            nc.sync.dma_start(out=out2a_o[:, :], in_=o2a_sb[:, :])
            o2b_sb = outp.tile([L, L], f32, tag="o2b_sb")
            nc.vector.tensor_copy(o2b_sb[:, :], o2b_ps[:, :])
            nc.sync.dma_start(out=out2b_o[:, :], in_=o2b_sb[:, :])

    nc.compile()
    return nc


def get_program():
    if "nc" not in _cache:
        _cache["nc"] = _build_program()
    return _cache["nc"]


def run_cores(inputs, **run_kwargs):
    """Build in_maps, run on 8 cores, return per-core result dicts."""
    from concourse.bass_utils import run_bass_kernel_spmd

    emb = np.asarray(inputs["emb"], dtype=np.float32)
    adj = np.asarray(inputs["adj"], dtype=np.float32)
    logits = np.asarray(inputs["logits_param"], dtype=np.float32)
    gumbel = np.asarray(inputs["gumbel_noise"], dtype=np.float32)

    logits_t1 = (logits / np.float32(T1)).astype(np.float32)
    logits_t2 = (logits / np.float32(T2)).astype(np.float32)

    def to_pnk(x):
        # [N, K] row-major -> [P, NB*K] where element (p, nb*K+k) = x[nb*P+p, k]
        return np.ascontiguousarray(
            x.reshape(NB, P, K).transpose(1, 0, 2).reshape(P, NB * K)
        )

    lt2_pnk = to_pnk(logits_t2)

    in_maps = []
    for c in range(8):
        b, h = c // 2, c % 2
        in_maps.append({
            "adj_cols": np.ascontiguousarray(adj[b, :, h * MH:(h + 1) * MH]),
            "emb_in": np.ascontiguousarray(emb[b, h * MH:(h + 1) * MH]),
            "z_in": to_pnk((logits_t1 + gumbel[b]).astype(np.float32)),
            "logits_t2": lt2_pnk,
        })

    nc = get_program()
    return run_bass_kernel_spmd(nc, in_maps, core_ids=list(range(8)), **run_kwargs)


def finalize(results):
    """Host-side unshard + tiny [K,K]-level finalization (fp32)."""
    out_emb = np.zeros((B, K, F), dtype=np.float32)
    out_adj = np.zeros((B, K, K), dtype=np.float32)
    s_sample = np.zeros((B, N, K), dtype=np.float32)
    num = np.zeros(B, dtype=np.float32)
    den = np.zeros(B, dtype=np.float32)

    for b in range(B):
        re, ro = results[2 * b], results[2 * b + 1]
        out2 = re["out2a_o"] + ro["out2b_o"]
        out_emb[b] = re["out_emb_a_o"] + ro["out_emb_b_o"]
        s_sample[b] = re["s_sample_o"]
        G = out2[0:K, 0:K].T
        num[b] = np.trace(out2[K:2 * K, K:2 * K])
        den[b] = out2[2 * K + 1, 2 * K]
        A = G * (np.float32(1.0) - np.eye(K, dtype=np.float32))
        d = np.sqrt(A.sum(axis=-1, keepdims=True)) + np.float32(EPS)
        out_adj[b] = A / d / d.T

    mincut_loss = np.float32(-np.mean(num / den))

    ss = results[0]["ss_o"]
    ss_n = ss / np.linalg.norm(ss)
    ortho_loss = np.float32(
        np.linalg.norm(ss_n - np.eye(K, dtype=np.float32) / np.float32(np.sqrt(K)))
    )
    return out_emb, out_adj, s_sample, mincut_loss, ortho_loss


def kernel(**inputs):
    res = run_cores(inputs)
    return finalize(res.results)
